# revision 2
# baseline (speedup 1.0000x reference)
"""Trainium2 Bass kernel v2 for nn_MultiHeadAttention_47485158424810.

Sharding (8 cores): core = b*4 + hg — data parallel over batch b, tensor
parallel over 4 head-groups (4 heads x 64 dims = 256 out dims per core).
Each core emits a partial [2048, 1024] f32 output; host sums 4 partials
per batch and adds (Wv_b @ Wo_w.T + Wo_b) — the V-bias is folded on host
since softmax rows sum to one.

v2 vs baseline: bf16 matmul operands (FWL weight loads, 2-4x DVE, half
input DMA; psum accumulate stays f32), attention output DMA'd straight
from PSUM into f32r oT tiles (no staging copies), denominators packed
[2, 2T] for one cheap reciprocal per chunk, per-chunk phase-3
(normalize + output projection + y DMA) interleaved under the next
chunk's attention, and far fewer DMA dispatches.
"""

import os
import sys

import numpy as np

for _p in ("/root/.axon_site/_ro/trn_rl_repo", "/opt/trn_rl_repo"):
    if os.path.isdir(_p) and _p not in sys.path:
        sys.path.append(_p)

import ml_dtypes

import concourse.bass as bass
import concourse.tile as tile
from concourse import bacc, mybir
from concourse.bass_utils import run_bass_kernel_spmd

B, T, K, H = 2, 2048, 1024, 16
NCORES = 8
O = 256  # head-group width per core (4 heads x 64)
S = 64  # head dim
HPC = 4  # heads per core
F32 = mybir.dt.float32
F32R = mybir.dt.float32r
BF16 = mybir.dt.bfloat16
AF = mybir.ActivationFunctionType
ALU = mybir.AluOpType
BF16NP = ml_dtypes.bfloat16

_CACHE = {}


def _build_body(nc, tc, d, loop_n=0):
    # pools live OUTSIDE the timing loop: iterations then pipeline into
    # each other instead of draining at per-iteration pool teardown
    with tc.tile_pool(name="consts", bufs=1) as consts, \
         tc.tile_pool(name="persist", bufs=1) as persist, \
         tc.tile_pool(name="pss", bufs=2, space="PSUM") as pss_p, \
         tc.tile_pool(name="ppr", bufs=2, space="PSUM") as ppr_p, \
         tc.tile_pool(name="pso", bufs=1, space="PSUM") as pso_p, \
         tc.tile_pool(name="ptile", bufs=6) as pt_p:
        pools = (consts, persist, pss_p, ppr_p, pso_p, pt_p)
        if loop_n:
            with tc.For_i(0, loop_n, 1):
                _build_inner(nc, tc, d, *pools)
        else:
            _build_inner(nc, tc, d, *pools)


def _build_inner(nc, tc, d, consts, persist, pss_p, ppr_p, pso_p, pt_p):
    f32 = F32
    x_d, wq_d, wk_d, wv_d, wo_d, bq_d, bk_d, y_d = (
        d["x"], d["wqT"], d["wkT"], d["wvT"], d["woT"],
        d["bq"], d["bk"], d["y"],
    )

    # ---- constant init first: no DMA deps, fills engines while DMAs land
    # causal partial-tile mask: [128, 2x128] bf16 (twin halves per head pair)
    trimask = consts.tile([128, 256], BF16, name="trimask")
    nc.gpsimd.memset(trimask, 1.0)
    tm3 = trimask.rearrange("p (e j) -> p e j", e=2)
    nc.gpsimd.affine_select(
        out=tm3, in_=tm3, pattern=[[0, 2], [1, 128]],
        compare_op=ALU.is_ge, fill=0.0, base=0, channel_multiplier=-1)

    # head-pair selector for the rank-1 denominator broadcast:
    # sel2[0, 0:64] = 1, sel2[1, 64:128] = 1
    sel2_f = consts.tile([2, 128], f32, name="sel2_f")
    sel2 = consts.tile([2, 128], BF16, name="sel2")
    nc.gpsimd.memset(sel2_f, 1.0)
    s3 = sel2_f.rearrange("p (e j) -> p e j", e=2)
    nc.gpsimd.affine_select(
        out=s3, in_=s3, pattern=[[1, 2], [0, 64]],
        compare_op=ALU.is_equal, fill=0.0, base=0, channel_multiplier=-1)
    nc.vector.tensor_copy(sel2, sel2_f)

    # ---- weights: one 3D DMA per tensor, packed [128, 8*256] (k-major) ----
    def load_w(ap_d, nm, eng):
        t_ = consts.tile([128, 8 * O], BF16, name=nm)
        eng.dma_start(t_.rearrange("p (kk o) -> p kk o", kk=8),
                      ap_d.rearrange("(kk p) o -> p kk o", p=128))
        return t_

    wq_sb = load_w(wq_d, "wq", nc.scalar)
    wk_sb = load_w(wk_d, "wk", nc.scalar)
    wv_sb = load_w(wv_d, "wv", nc.scalar)
    wo_sb = []
    for oc in range(2):
        t_ = consts.tile([128, K], BF16, name=f"wo{oc}")
        nc.gpsimd.dma_start(t_, wo_d[oc * 128:(oc + 1) * 128, :])
        wo_sb.append(t_)

    def load_bias(ap_d, nm):
        t_ = consts.tile([128, 2], f32, name=nm)
        nc.gpsimd.dma_start(t_, ap_d.rearrange("(c p) -> p c", p=128))
        return t_

    bq_sb = load_bias(bq_d, "bq_sb")
    bk_sb = load_bias(bk_d, "bk_sb")

    # x^T, full T per k-slice.  Chunk-0 slices first (they gate the first
    # projection), then the remainder.
    xT = [persist.tile([128, T], BF16, name=f"xT{kk}") for kk in range(8)]
    qs = (nc.sync, nc.gpsimd)
    for kk in range(8):
        qs[kk % 2].dma_start(xT[kk][:, 0:512],
                             x_d[kk * 128:(kk + 1) * 128, 0:512])
    for kk in range(8):
        qs[kk % 2].dma_start(xT[kk][:, 512:T],
                             x_d[kk * 128:(kk + 1) * 128, 512:T])

    # persistent activations
    qT = [persist.tile([128, T], BF16, name=f"qT{oc}") for oc in range(2)]
    kT = [persist.tile([128, T], BF16, name=f"kT{oc}") for oc in range(2)]
    oT = [persist.tile([128, T], BF16, name=f"oT{oc}") for oc in range(2)]
    # V natural layout per 128-token tile: 4 heads x (64 dims + ones col)
    vv = [persist.tile([128, HPC * (S + 1)], BF16, name=f"v{i}")
          for i in range(T // 128)]
    # softmax denominators / reciprocals: row e (head-in-pair), col oc*T + t
    rsum = persist.tile([2, 2 * T], BF16, name="rsum")
    rrec = persist.tile([2, 2 * T], BF16, name="rrec")
    ones_f32 = persist.tile([128, 4], f32, name="ones_f32")
    nc.gpsimd.memset(ones_f32, 1.0)
    for i in range(T // 128):
        # ones column at offset h*(S+1)+S for each head
        nc.vector.tensor_copy(vv[i][:, S::S + 1], ones_f32)

    inv_scale = 1.0 / float(np.sqrt(K))

    # steady-state loop: proj(c) -> attention(c) -> phase3(c-1), with the
    # Tile scheduler overlapping phase3(c-1) + proj(c+1) under attention.
    # PSUM: pss 2x[128,1024] (4 banks) + po 2x[65,512] (2) + ppr shared
    # proj/prb/py tag (2) = 8 banks exactly.
    def phase3(c, pool, ys_pool):
        # reciprocal of denominators for this chunk's tokens (both oc)
        cr = slice(c * 512, (c + 1) * 512)
        rs3 = rsum.rearrange("p (oc t) -> p oc t", oc=2)[:, :, cr]
        rr3 = rrec.rearrange("p (oc t) -> p oc t", oc=2)[:, :, cr]
        with nc.allow_low_precision(reason="softmax denom reciprocal"):
            nc.vector.reciprocal(rr3, rs3)
        for oc in range(2):
            prb = pool.tile([128, 512], f32, name="prb", tag="ps")
            nc.tensor.matmul(
                prb, sel2, rrec[:, oc * T + c * 512:oc * T + (c + 1) * 512],
                start=True, stop=True)
            nc.vector.tensor_mul(oT[oc][:, cr], oT[oc][:, cr], prb)
        for i in range(4 * c, 4 * c + 4):
            ys = ys_pool.tile([128, K], BF16, name="ystg", tag="ystg",
                              bufs=2)
            for jc in range(2):
                py = pool.tile([128, 512], f32, name="py", tag="ps")
                for oc in range(2):
                    nc.tensor.matmul(
                        py,
                        oT[oc][:, i * 128:(i + 1) * 128],
                        wo_sb[oc][:, jc * 512:(jc + 1) * 512],
                        start=(oc == 0), stop=(oc == 1))
                nc.vector.tensor_copy(ys[:, jc * 512:(jc + 1) * 512], py)
            nc.sync.dma_start(y_d[i * 128:(i + 1) * 128, :], ys)

    if True:
        for c in range(4):  # chunks of 512 tokens
            tch = slice(c * 512, (c + 1) * 512)
            # ---- projections for chunk c ----
            # Q^T / K^T: [o on partitions, t free]
            for w_sb, b_sb, dest in ((wq_sb, bq_sb, qT), (wk_sb, bk_sb, kT)):
                for oc in range(2):
                    ps = ppr_p.tile([128, 512], f32, name="ps_qk", tag="ps")
                    for kk in range(8):
                        nc.tensor.matmul(
                            ps,
                            w_sb[:, kk * O + oc * 128:kk * O + (oc + 1) * 128],
                            xT[kk][:, tch],
                            start=(kk == 0), stop=(kk == 7))
                    nc.vector.tensor_scalar_add(
                        dest[oc][:, tch], ps, b_sb[:, oc:oc + 1])
            # V natural: [t on partitions, o free]; two 128-token tiles per
            # psum alloc; no bias (folded on host)
            for ah in range(2):
                ps = ppr_p.tile([128, 512], f32, name="ps_v", tag="ps")
                for a2 in range(2):
                    a = 2 * ah + a2
                    for kk in range(8):
                        nc.tensor.matmul(
                            ps[:, a2 * O:(a2 + 1) * O],
                            xT[kk][:, c * 512 + a * 128:c * 512 + (a + 1) * 128],
                            wv_sb[:, kk * O:(kk + 1) * O],
                            start=(kk == 0), stop=(kk == 7))
                for a2 in range(2):
                    a = 2 * ah + a2
                    nc.vector.tensor_copy(
                        vv[c * 4 + a].rearrange(
                            "p (h x) -> p h x", h=HPC)[:, :, 0:S],
                        ps.rearrange("p (a2 h x) -> p a2 h x",
                                     a2=2, h=HPC)[:, a2])

            # ---- attention for q-chunk c ----
            for oc in range(2):  # head pair (2*oc, 2*oc+1)
                po = [pso_p.tile([S + 1, 512], f32, name=f"po{e}",
                                 tag=f"po{e}") for e in range(2)]
                nr = 4 * (c + 1)  # causal: t_k tiles 0..4c+3
                for r in range(nr):
                    m = r - 4 * c
                    j0 = 128 * m if m > 0 else 0  # fully-masked cols skipped
                    ps = pss_p.tile([128, 1024], f32, name="ps_s", tag="pss")
                    for e in range(2):
                        hb = e * 64
                        nc.tensor.matmul(
                            ps[:, e * 512 + j0:(e + 1) * 512],
                            kT[oc][hb:hb + 64, r * 128:(r + 1) * 128],
                            qT[oc][hb:hb + 64, c * 512 + j0:(c + 1) * 512],
                            start=True, stop=True)
                    pt = pt_p.tile([128, 1024], BF16, name="pt_exp",
                                   tag="ptl", bufs=6)
                    ps3 = ps.rearrange("p (e j) -> p e j", e=2)[:, :, j0:]
                    pt3 = pt.rearrange("p (e j) -> p e j", e=2)[:, :, j0:]
                    nc.scalar.activation(pt3, ps3, AF.Exp, scale=inv_scale)
                    if m >= 0:
                        # only the first 128 trimmed columns are partial
                        nc.vector.tensor_mul(
                            pt3[:, :, 0:128], pt3[:, :, 0:128],
                            trimask.rearrange("p (e j) -> p e j", e=2))
                    for e in range(2):
                        h = 2 * oc + e
                        nc.tensor.matmul(
                            po[e][:, j0:],
                            vv[r][:, h * (S + 1):(h + 1) * (S + 1)],
                            pt[:, e * 512 + j0:(e + 1) * 512],
                            start=(r == 0), stop=(r == nr - 1))
                # evict unnormalized O^T rows + denominator row via bf16
                # staging (DMA cannot read PSUM)
                for e in range(2):
                    hb = e * 64
                    stg = pt_p.tile([S + 1, 512], BF16, name=f"ostg{e}",
                                    tag=f"ostg{e}", bufs=2)
                    nc.vector.tensor_copy(stg, po[e])
                    nc.gpsimd.dma_start(oT[oc][hb:hb + 64, tch], stg[0:S, :])
                    nc.sync.dma_start(
                        rsum[e:e + 1, oc * T + c * 512:oc * T + (c + 1) * 512],
                        stg[S:S + 1, :])

            if c > 0:
                phase3(c - 1, ppr_p, pt_p)
        phase3(3, ppr_p, pt_p)


def build_program(loop_n=0):
    nc = bacc.Bacc("TRN2", target_bir_lowering=False, debug=False,
                   num_devices=NCORES)
    d = {
        "x": nc.dram_tensor("xT", [K, T], BF16, kind="ExternalInput").ap(),
        "wqT": nc.dram_tensor("wqT", [K, O], BF16, kind="ExternalInput").ap(),
        "wkT": nc.dram_tensor("wkT", [K, O], BF16, kind="ExternalInput").ap(),
        "wvT": nc.dram_tensor("wvT", [K, O], BF16, kind="ExternalInput").ap(),
        "woT": nc.dram_tensor("woT", [O, K], BF16, kind="ExternalInput").ap(),
        "bq": nc.dram_tensor("bq", [O], F32, kind="ExternalInput").ap(),
        "bk": nc.dram_tensor("bk", [O], F32, kind="ExternalInput").ap(),
        "y": nc.dram_tensor("y", [T, K], BF16, kind="ExternalOutput").ap(),
    }
    with tile.TileContext(nc) as tc:
        _build_body(nc, tc, d, loop_n=loop_n)
    nc.compile()
    return nc


def _get_program():
    if "nc" not in _CACHE:
        _CACHE["nc"] = build_program()
    return _CACHE["nc"]


def make_in_maps(x, Wq_w, Wk_w, Wv_w, Wo_w, Wq_b, Wk_b, Wv_b):
    in_maps = []
    for core in range(NCORES):
        b, hg = divmod(core, 4)
        sl = slice(hg * O, (hg + 1) * O)
        in_maps.append({
            "xT": np.ascontiguousarray(np.asarray(x[b]).T).astype(BF16NP),
            "wqT": np.ascontiguousarray(np.asarray(Wq_w)[sl, :].T).astype(BF16NP),
            "wkT": np.ascontiguousarray(np.asarray(Wk_w)[sl, :].T).astype(BF16NP),
            "wvT": np.ascontiguousarray(np.asarray(Wv_w)[sl, :].T).astype(BF16NP),
            "woT": np.ascontiguousarray(np.asarray(Wo_w)[:, sl].T).astype(BF16NP),
            "bq": np.ascontiguousarray(np.asarray(Wq_b)[sl], np.float32),
            "bk": np.ascontiguousarray(np.asarray(Wk_b)[sl], np.float32),
        })
    return in_maps


def _combine(results, Wv_b, Wo_w, Wo_b):
    bias_row = (np.asarray(Wv_b, np.float32) @ np.asarray(Wo_w, np.float32).T
                + np.asarray(Wo_b, np.float32))
    y = np.empty((B, T, K), np.float32)
    for b in range(B):
        acc = np.asarray(results[b * 4]["y"], np.float32)
        for hg in range(1, 4):
            acc = acc + np.asarray(results[b * 4 + hg]["y"], np.float32)
        y[b] = acc + bias_row
    return y


def kernel(x, Wq_w, Wq_b, Wk_w, Wk_b, Wv_w, Wv_b, Wo_w, Wo_b):
    x = np.asarray(x, np.float32)
    nc = _get_program()
    in_maps = make_in_maps(x, Wq_w, Wk_w, Wv_w, Wo_w, Wq_b, Wk_b, Wv_b)
    out = run_bass_kernel_spmd(nc, in_maps, list(range(NCORES)))
    return _combine(out.results, Wv_b, Wo_w, Wo_b)


# revision 3
# speedup vs baseline: 1.1170x; 1.1170x over previous
"""Trainium2 Bass kernel v2 for nn_MultiHeadAttention_47485158424810.

Sharding (8 cores): core = b*4 + hg — data parallel over batch b, tensor
parallel over 4 head-groups (4 heads x 64 dims = 256 out dims per core).
Each core emits a partial [2048, 1024] f32 output; host sums 4 partials
per batch and adds (Wv_b @ Wo_w.T + Wo_b) — the V-bias is folded on host
since softmax rows sum to one.

vs the f32r baseline (275us): bf16 matmul operands everywhere with f32
PSUM accumulation (FWL weight loads, 2-4x DVE throughput, half the
DMA bytes), denominators packed [2, 2T] so one cheap reciprocal per
chunk replaces four single-partition ones, the rank-1 denominator
broadcast done as a single contraction-2 matmul per (chunk, pair),
V-bias folded into the host combine, per-chunk phase-3 (normalize +
output projection + y DMA) interleaved under the next chunk's
attention via a shared PSUM pool tag (8 banks exactly), and fewer,
queue-balanced DMA dispatches.  Measured 187.2us on HW (loop-delta),
rel err 5.0e-3.
"""

import os
import sys

import numpy as np

for _p in ("/root/.axon_site/_ro/trn_rl_repo", "/opt/trn_rl_repo"):
    if os.path.isdir(_p) and _p not in sys.path:
        sys.path.append(_p)

import ml_dtypes

import concourse.bass as bass
import concourse.tile as tile
from concourse import bacc, mybir
from concourse.bass_utils import run_bass_kernel_spmd

B, T, K, H = 2, 2048, 1024, 16
NCORES = 8
O = 256  # head-group width per core (4 heads x 64)
S = 64  # head dim
HPC = 4  # heads per core
F32 = mybir.dt.float32
F32R = mybir.dt.float32r
BF16 = mybir.dt.bfloat16
AF = mybir.ActivationFunctionType
ALU = mybir.AluOpType
BF16NP = ml_dtypes.bfloat16

_CACHE = {}


def _build_body(nc, tc, d, loop_n=0):
    # pools live OUTSIDE the timing loop: iterations then pipeline into
    # each other instead of draining at per-iteration pool teardown
    with tc.tile_pool(name="consts", bufs=1) as consts, \
         tc.tile_pool(name="persist", bufs=1) as persist, \
         tc.tile_pool(name="pss", bufs=2, space="PSUM") as pss_p, \
         tc.tile_pool(name="ppr", bufs=2, space="PSUM") as ppr_p, \
         tc.tile_pool(name="pso", bufs=1, space="PSUM") as pso_p, \
         tc.tile_pool(name="ptile", bufs=6) as pt_p:
        pools = (consts, persist, pss_p, ppr_p, pso_p, pt_p)
        if loop_n:
            with tc.For_i(0, loop_n, 1):
                _build_inner(nc, tc, d, *pools)
        else:
            _build_inner(nc, tc, d, *pools)


def _build_inner(nc, tc, d, consts, persist, pss_p, ppr_p, pso_p, pt_p):
    f32 = F32
    x_d, wq_d, wk_d, wv_d, wo_d, bq_d, bk_d, y_d = (
        d["x"], d["wqT"], d["wkT"], d["wvT"], d["woT"],
        d["bq"], d["bk"], d["y"],
    )

    # ---- constant init first: no DMA deps, fills engines while DMAs land
    # causal partial-tile mask: [128, 2x128] bf16 (twin halves per head pair)
    trimask = consts.tile([128, 256], BF16, name="trimask")
    nc.gpsimd.memset(trimask, 1.0)
    tm3 = trimask.rearrange("p (e j) -> p e j", e=2)
    nc.gpsimd.affine_select(
        out=tm3, in_=tm3, pattern=[[0, 2], [1, 128]],
        compare_op=ALU.is_ge, fill=0.0, base=0, channel_multiplier=-1)

    # head-pair selector for the rank-1 denominator broadcast:
    # sel2[0, 0:64] = 1, sel2[1, 64:128] = 1
    sel2_f = consts.tile([2, 128], f32, name="sel2_f")
    sel2 = consts.tile([2, 128], BF16, name="sel2")
    nc.gpsimd.memset(sel2_f, 1.0)
    s3 = sel2_f.rearrange("p (e j) -> p e j", e=2)
    nc.gpsimd.affine_select(
        out=s3, in_=s3, pattern=[[1, 2], [0, 64]],
        compare_op=ALU.is_equal, fill=0.0, base=0, channel_multiplier=-1)
    nc.vector.tensor_copy(sel2, sel2_f)

    # ---- weights: one 3D DMA per tensor, packed [128, 8*256] (k-major) ----
    def load_w(ap_d, nm, eng):
        t_ = consts.tile([128, 8 * O], BF16, name=nm)
        eng.dma_start(t_.rearrange("p (kk o) -> p kk o", kk=8),
                      ap_d.rearrange("(kk p) o -> p kk o", p=128))
        return t_

    wq_sb = load_w(wq_d, "wq", nc.scalar)
    wk_sb = load_w(wk_d, "wk", nc.scalar)
    wv_sb = load_w(wv_d, "wv", nc.scalar)
    wo_sb = []
    for oc in range(2):
        t_ = consts.tile([128, K], BF16, name=f"wo{oc}")
        nc.gpsimd.dma_start(t_, wo_d[oc * 128:(oc + 1) * 128, :])
        wo_sb.append(t_)

    def load_bias(ap_d, nm):
        t_ = consts.tile([128, 2], f32, name=nm)
        nc.gpsimd.dma_start(t_, ap_d.rearrange("(c p) -> p c", p=128))
        return t_

    bq_sb = load_bias(bq_d, "bq_sb")
    bk_sb = load_bias(bk_d, "bk_sb")

    # x^T, full T per k-slice.  Chunk-0 slices first (they gate the first
    # projection), then the remainder.
    xT = [persist.tile([128, T], BF16, name=f"xT{kk}") for kk in range(8)]
    qs = (nc.sync, nc.gpsimd)
    for kk in range(8):
        qs[kk % 2].dma_start(xT[kk][:, 0:512],
                             x_d[kk * 128:(kk + 1) * 128, 0:512])
    for kk in range(8):
        qs[kk % 2].dma_start(xT[kk][:, 512:T],
                             x_d[kk * 128:(kk + 1) * 128, 512:T])

    # persistent activations
    qT = [persist.tile([128, T], BF16, name=f"qT{oc}") for oc in range(2)]
    kT = [persist.tile([128, T], BF16, name=f"kT{oc}") for oc in range(2)]
    oT = [persist.tile([128, T], BF16, name=f"oT{oc}") for oc in range(2)]
    # V natural layout per 128-token tile: 4 heads x (64 dims + ones col)
    vv = [persist.tile([128, HPC * (S + 1)], BF16, name=f"v{i}")
          for i in range(T // 128)]
    # softmax denominators / reciprocals: row e (head-in-pair), col oc*T + t
    rsum = persist.tile([2, 2 * T], BF16, name="rsum")
    rrec = persist.tile([2, 2 * T], BF16, name="rrec")
    ones_f32 = persist.tile([128, 4], f32, name="ones_f32")
    nc.gpsimd.memset(ones_f32, 1.0)
    for i in range(T // 128):
        # ones column at offset h*(S+1)+S for each head
        nc.vector.tensor_copy(vv[i][:, S::S + 1], ones_f32)

    inv_scale = 1.0 / float(np.sqrt(K))

    # steady-state loop: proj(c) -> attention(c) -> phase3(c-1), with the
    # Tile scheduler overlapping phase3(c-1) + proj(c+1) under attention.
    # PSUM: pss 2x[128,1024] (4 banks) + po 2x[65,512] (2) + ppr shared
    # proj/prb/py tag (2) = 8 banks exactly.
    def phase3(c, pool, ys_pool):
        # reciprocal of denominators for this chunk's tokens (both oc)
        cr = slice(c * 512, (c + 1) * 512)
        rs3 = rsum.rearrange("p (oc t) -> p oc t", oc=2)[:, :, cr]
        rr3 = rrec.rearrange("p (oc t) -> p oc t", oc=2)[:, :, cr]
        with nc.allow_low_precision(reason="softmax denom reciprocal"):
            nc.vector.reciprocal(rr3, rs3)
        for oc in range(2):
            prb = pool.tile([128, 512], f32, name="prb", tag="ps")
            nc.tensor.matmul(
                prb, sel2, rrec[:, oc * T + c * 512:oc * T + (c + 1) * 512],
                start=True, stop=True)
            nc.vector.tensor_mul(oT[oc][:, cr], oT[oc][:, cr], prb)
        for i in range(4 * c, 4 * c + 4):
            ys = ys_pool.tile([128, K], BF16, name="ystg", tag="ystg",
                              bufs=2)
            for jc in range(2):
                py = pool.tile([128, 512], f32, name="py", tag="ps")
                for oc in range(2):
                    nc.tensor.matmul(
                        py,
                        oT[oc][:, i * 128:(i + 1) * 128],
                        wo_sb[oc][:, jc * 512:(jc + 1) * 512],
                        start=(oc == 0), stop=(oc == 1))
                nc.vector.tensor_copy(ys[:, jc * 512:(jc + 1) * 512], py)
            nc.sync.dma_start(y_d[i * 128:(i + 1) * 128, :], ys)

    if True:
        for c in range(4):  # chunks of 512 tokens
            tch = slice(c * 512, (c + 1) * 512)
            # ---- projections for chunk c ----
            # Q^T / K^T: [o on partitions, t free]
            for w_sb, b_sb, dest in ((wq_sb, bq_sb, qT), (wk_sb, bk_sb, kT)):
                for oc in range(2):
                    ps = ppr_p.tile([128, 512], f32, name="ps_qk", tag="ps")
                    for kk in range(8):
                        nc.tensor.matmul(
                            ps,
                            w_sb[:, kk * O + oc * 128:kk * O + (oc + 1) * 128],
                            xT[kk][:, tch],
                            start=(kk == 0), stop=(kk == 7))
                    nc.vector.tensor_scalar_add(
                        dest[oc][:, tch], ps, b_sb[:, oc:oc + 1])
            # V natural: [t on partitions, o free]; two 128-token tiles per
            # psum alloc; no bias (folded on host)
            for ah in range(2):
                ps = ppr_p.tile([128, 512], f32, name="ps_v", tag="ps")
                for a2 in range(2):
                    a = 2 * ah + a2
                    for kk in range(8):
                        nc.tensor.matmul(
                            ps[:, a2 * O:(a2 + 1) * O],
                            xT[kk][:, c * 512 + a * 128:c * 512 + (a + 1) * 128],
                            wv_sb[:, kk * O:(kk + 1) * O],
                            start=(kk == 0), stop=(kk == 7))
                for a2 in range(2):
                    a = 2 * ah + a2
                    nc.vector.tensor_copy(
                        vv[c * 4 + a].rearrange(
                            "p (h x) -> p h x", h=HPC)[:, :, 0:S],
                        ps.rearrange("p (a2 h x) -> p a2 h x",
                                     a2=2, h=HPC)[:, a2])

            # ---- attention for q-chunk c ----
            for oc in range(2):  # head pair (2*oc, 2*oc+1)
                po = [pso_p.tile([S + 1, 512], f32, name=f"po{e}",
                                 tag=f"po{e}") for e in range(2)]
                nr = 4 * (c + 1)  # causal: t_k tiles 0..4c+3
                for r in range(nr):
                    m = r - 4 * c
                    j0 = 128 * m if m > 0 else 0  # fully-masked cols skipped
                    ps = pss_p.tile([128, 1024], f32, name="ps_s", tag="pss")
                    for e in range(2):
                        hb = e * 64
                        nc.tensor.matmul(
                            ps[:, e * 512 + j0:(e + 1) * 512],
                            kT[oc][hb:hb + 64, r * 128:(r + 1) * 128],
                            qT[oc][hb:hb + 64, c * 512 + j0:(c + 1) * 512],
                            start=True, stop=True)
                    pt = pt_p.tile([128, 1024], BF16, name="pt_exp",
                                   tag="ptl", bufs=6)
                    ps3 = ps.rearrange("p (e j) -> p e j", e=2)[:, :, j0:]
                    pt3 = pt.rearrange("p (e j) -> p e j", e=2)[:, :, j0:]
                    nc.scalar.activation(pt3, ps3, AF.Exp, scale=inv_scale)
                    if m >= 0:
                        # only the first 128 trimmed columns are partial
                        nc.vector.tensor_mul(
                            pt3[:, :, 0:128], pt3[:, :, 0:128],
                            trimask.rearrange("p (e j) -> p e j", e=2))
                    for e in range(2):
                        h = 2 * oc + e
                        nc.tensor.matmul(
                            po[e][:, j0:],
                            vv[r][:, h * (S + 1):(h + 1) * (S + 1)],
                            pt[:, e * 512 + j0:(e + 1) * 512],
                            start=(r == 0), stop=(r == nr - 1))
                # evict unnormalized O^T rows + denominator row via bf16
                # staging (DMA cannot read PSUM)
                for e in range(2):
                    hb = e * 64
                    stg = pt_p.tile([S + 1, 512], BF16, name=f"ostg{e}",
                                    tag=f"ostg{e}", bufs=2)
                    nc.vector.tensor_copy(stg, po[e])
                    nc.gpsimd.dma_start(oT[oc][hb:hb + 64, tch], stg[0:S, :])
                    nc.sync.dma_start(
                        rsum[e:e + 1, oc * T + c * 512:oc * T + (c + 1) * 512],
                        stg[S:S + 1, :])

            if c > 0:
                phase3(c - 1, ppr_p, pt_p)
        phase3(3, ppr_p, pt_p)


def build_program(loop_n=0):
    nc = bacc.Bacc("TRN2", target_bir_lowering=False, debug=False,
                   num_devices=NCORES)
    d = {
        "x": nc.dram_tensor("xT", [K, T], BF16, kind="ExternalInput").ap(),
        "wqT": nc.dram_tensor("wqT", [K, O], BF16, kind="ExternalInput").ap(),
        "wkT": nc.dram_tensor("wkT", [K, O], BF16, kind="ExternalInput").ap(),
        "wvT": nc.dram_tensor("wvT", [K, O], BF16, kind="ExternalInput").ap(),
        "woT": nc.dram_tensor("woT", [O, K], BF16, kind="ExternalInput").ap(),
        "bq": nc.dram_tensor("bq", [O], F32, kind="ExternalInput").ap(),
        "bk": nc.dram_tensor("bk", [O], F32, kind="ExternalInput").ap(),
        "y": nc.dram_tensor("y", [T, K], BF16, kind="ExternalOutput").ap(),
    }
    with tile.TileContext(nc) as tc:
        _build_body(nc, tc, d, loop_n=loop_n)
    nc.compile()
    return nc


def _get_program():
    if "nc" not in _CACHE:
        _CACHE["nc"] = build_program()
    return _CACHE["nc"]


def make_in_maps(x, Wq_w, Wk_w, Wv_w, Wo_w, Wq_b, Wk_b, Wv_b):
    in_maps = []
    for core in range(NCORES):
        b, hg = divmod(core, 4)
        sl = slice(hg * O, (hg + 1) * O)
        in_maps.append({
            "xT": np.ascontiguousarray(np.asarray(x[b]).T).astype(BF16NP),
            "wqT": np.ascontiguousarray(np.asarray(Wq_w)[sl, :].T).astype(BF16NP),
            "wkT": np.ascontiguousarray(np.asarray(Wk_w)[sl, :].T).astype(BF16NP),
            "wvT": np.ascontiguousarray(np.asarray(Wv_w)[sl, :].T).astype(BF16NP),
            "woT": np.ascontiguousarray(np.asarray(Wo_w)[:, sl].T).astype(BF16NP),
            "bq": np.ascontiguousarray(np.asarray(Wq_b)[sl], np.float32),
            "bk": np.ascontiguousarray(np.asarray(Wk_b)[sl], np.float32),
        })
    return in_maps


def _combine(results, Wv_b, Wo_w, Wo_b):
    bias_row = (np.asarray(Wv_b, np.float32) @ np.asarray(Wo_w, np.float32).T
                + np.asarray(Wo_b, np.float32))
    y = np.empty((B, T, K), np.float32)
    for b in range(B):
        acc = np.asarray(results[b * 4]["y"], np.float32)
        for hg in range(1, 4):
            acc = acc + np.asarray(results[b * 4 + hg]["y"], np.float32)
        y[b] = acc + bias_row
    return y


def kernel(x, Wq_w, Wq_b, Wk_w, Wk_b, Wv_w, Wv_b, Wo_w, Wo_b):
    x = np.asarray(x, np.float32)
    nc = _get_program()
    in_maps = make_in_maps(x, Wq_w, Wk_w, Wv_w, Wo_w, Wq_b, Wk_b, Wv_b)
    out = run_bass_kernel_spmd(nc, in_maps, list(range(NCORES)))
    return _combine(out.results, Wv_b, Wo_w, Wo_b)


# revision 4
# speedup vs baseline: 1.1577x; 1.0364x over previous
"""Trainium2 Bass kernel v2 for nn_MultiHeadAttention_47485158424810.

Sharding (8 cores): core = b*4 + hg — data parallel over batch b, tensor
parallel over 4 head-groups (4 heads x 64 dims = 256 out dims per core).
Each core emits a partial [2048, 1024] f32 output; host sums 4 partials
per batch and adds (Wv_b @ Wo_w.T + Wo_b) — the V-bias is folded on host
since softmax rows sum to one.

vs the f32r baseline (275us): bf16 matmul operands everywhere with f32
PSUM accumulation (FWL weight loads, 2-4x DVE throughput, half the
DMA bytes), denominators packed [2, 2T] so one cheap reciprocal per
chunk replaces four single-partition ones, the rank-1 denominator
broadcast done as a single contraction-2 matmul per (chunk, pair),
V-bias folded into the host combine, per-chunk phase-3 (normalize +
output projection + y DMA) interleaved under the next chunk's
attention via a shared PSUM pool tag (8 banks exactly), and fewer,
queue-balanced DMA dispatches, with deep staging pools (pt x8,
stg x3, ys x3) for pipeline elasticity.  Measured 180.9us on HW
(loop-delta), rel err 5.0e-3.
"""

import os
import sys

import numpy as np

for _p in ("/root/.axon_site/_ro/trn_rl_repo", "/opt/trn_rl_repo"):
    if os.path.isdir(_p) and _p not in sys.path:
        sys.path.append(_p)

import ml_dtypes

import concourse.bass as bass
import concourse.tile as tile
from concourse import bacc, mybir
from concourse.bass_utils import run_bass_kernel_spmd

B, T, K, H = 2, 2048, 1024, 16
NCORES = 8
O = 256  # head-group width per core (4 heads x 64)
S = 64  # head dim
HPC = 4  # heads per core
F32 = mybir.dt.float32
F32R = mybir.dt.float32r
BF16 = mybir.dt.bfloat16
AF = mybir.ActivationFunctionType
ALU = mybir.AluOpType
BF16NP = ml_dtypes.bfloat16

_CACHE = {}


def _build_body(nc, tc, d, loop_n=0):
    # pools live OUTSIDE the timing loop: iterations then pipeline into
    # each other instead of draining at per-iteration pool teardown
    with tc.tile_pool(name="consts", bufs=1) as consts, \
         tc.tile_pool(name="persist", bufs=1) as persist, \
         tc.tile_pool(name="pss", bufs=2, space="PSUM") as pss_p, \
         tc.tile_pool(name="ppr", bufs=2, space="PSUM") as ppr_p, \
         tc.tile_pool(name="pso", bufs=1, space="PSUM") as pso_p, \
         tc.tile_pool(name="ptile", bufs=6) as pt_p:
        pools = (consts, persist, pss_p, ppr_p, pso_p, pt_p)
        if loop_n:
            with tc.For_i(0, loop_n, 1):
                _build_inner(nc, tc, d, *pools)
        else:
            _build_inner(nc, tc, d, *pools)


def _build_inner(nc, tc, d, consts, persist, pss_p, ppr_p, pso_p, pt_p):
    f32 = F32
    x_d, wq_d, wk_d, wv_d, wo_d, bq_d, bk_d, y_d = (
        d["x"], d["wqT"], d["wkT"], d["wvT"], d["woT"],
        d["bq"], d["bk"], d["y"],
    )

    # ---- constant init first: no DMA deps, fills engines while DMAs land
    # causal partial-tile mask: [128, 2x128] bf16 (twin halves per head pair)
    trimask = consts.tile([128, 256], BF16, name="trimask")
    nc.gpsimd.memset(trimask, 1.0)
    tm3 = trimask.rearrange("p (e j) -> p e j", e=2)
    nc.gpsimd.affine_select(
        out=tm3, in_=tm3, pattern=[[0, 2], [1, 128]],
        compare_op=ALU.is_ge, fill=0.0, base=0, channel_multiplier=-1)

    # head-pair selector for the rank-1 denominator broadcast:
    # sel2[0, 0:64] = 1, sel2[1, 64:128] = 1
    sel2_f = consts.tile([2, 128], f32, name="sel2_f")
    sel2 = consts.tile([2, 128], BF16, name="sel2")
    nc.gpsimd.memset(sel2_f, 1.0)
    s3 = sel2_f.rearrange("p (e j) -> p e j", e=2)
    nc.gpsimd.affine_select(
        out=s3, in_=s3, pattern=[[1, 2], [0, 64]],
        compare_op=ALU.is_equal, fill=0.0, base=0, channel_multiplier=-1)
    nc.vector.tensor_copy(sel2, sel2_f)

    # ---- weights: one 3D DMA per tensor, packed [128, 8*256] (k-major) ----
    def load_w(ap_d, nm, eng):
        t_ = consts.tile([128, 8 * O], BF16, name=nm)
        eng.dma_start(t_.rearrange("p (kk o) -> p kk o", kk=8),
                      ap_d.rearrange("(kk p) o -> p kk o", p=128))
        return t_

    wq_sb = load_w(wq_d, "wq", nc.scalar)
    wk_sb = load_w(wk_d, "wk", nc.scalar)
    wv_sb = load_w(wv_d, "wv", nc.scalar)
    wo_sb = []
    for oc in range(2):
        t_ = consts.tile([128, K], BF16, name=f"wo{oc}")
        nc.gpsimd.dma_start(t_, wo_d[oc * 128:(oc + 1) * 128, :])
        wo_sb.append(t_)

    def load_bias(ap_d, nm):
        t_ = consts.tile([128, 2], f32, name=nm)
        nc.gpsimd.dma_start(t_, ap_d.rearrange("(c p) -> p c", p=128))
        return t_

    bq_sb = load_bias(bq_d, "bq_sb")
    bk_sb = load_bias(bk_d, "bk_sb")

    # x^T, full T per k-slice.  Chunk-0 slices first (they gate the first
    # projection), then the remainder.
    xT = [persist.tile([128, T], BF16, name=f"xT{kk}") for kk in range(8)]
    qs = (nc.sync, nc.gpsimd)
    for kk in range(8):
        qs[kk % 2].dma_start(xT[kk][:, 0:512],
                             x_d[kk * 128:(kk + 1) * 128, 0:512])
    for kk in range(8):
        qs[kk % 2].dma_start(xT[kk][:, 512:T],
                             x_d[kk * 128:(kk + 1) * 128, 512:T])

    # persistent activations
    qT = [persist.tile([128, T], BF16, name=f"qT{oc}") for oc in range(2)]
    kT = [persist.tile([128, T], BF16, name=f"kT{oc}") for oc in range(2)]
    oT = [persist.tile([128, T], BF16, name=f"oT{oc}") for oc in range(2)]
    # V natural layout per 128-token tile: 4 heads x (64 dims + ones col)
    vv = [persist.tile([128, HPC * (S + 1)], BF16, name=f"v{i}")
          for i in range(T // 128)]
    # softmax denominators / reciprocals: row e (head-in-pair), col oc*T + t
    rsum = persist.tile([2, 2 * T], BF16, name="rsum")
    rrec = persist.tile([2, 2 * T], BF16, name="rrec")
    ones_f32 = persist.tile([128, 4], f32, name="ones_f32")
    nc.gpsimd.memset(ones_f32, 1.0)
    for i in range(T // 128):
        # ones column at offset h*(S+1)+S for each head
        nc.vector.tensor_copy(vv[i][:, S::S + 1], ones_f32)

    inv_scale = 1.0 / float(np.sqrt(K))

    # steady-state loop: proj(c) -> attention(c) -> phase3(c-1), with the
    # Tile scheduler overlapping phase3(c-1) + proj(c+1) under attention.
    # PSUM: pss 2x[128,1024] (4 banks) + po 2x[65,512] (2) + ppr shared
    # proj/prb/py tag (2) = 8 banks exactly.
    def phase3(c, pool, ys_pool):
        # reciprocal of denominators for this chunk's tokens (both oc)
        cr = slice(c * 512, (c + 1) * 512)
        rs3 = rsum.rearrange("p (oc t) -> p oc t", oc=2)[:, :, cr]
        rr3 = rrec.rearrange("p (oc t) -> p oc t", oc=2)[:, :, cr]
        with nc.allow_low_precision(reason="softmax denom reciprocal"):
            nc.vector.reciprocal(rr3, rs3)
        for oc in range(2):
            prb = pool.tile([128, 512], f32, name="prb", tag="ps")
            nc.tensor.matmul(
                prb, sel2, rrec[:, oc * T + c * 512:oc * T + (c + 1) * 512],
                start=True, stop=True)
            nc.vector.tensor_mul(oT[oc][:, cr], oT[oc][:, cr], prb)
        for i in range(4 * c, 4 * c + 4):
            ys = ys_pool.tile([128, K], BF16, name="ystg", tag="ystg",
                              bufs=3)
            for jc in range(2):
                py = pool.tile([128, 512], f32, name="py", tag="ps")
                for oc in range(2):
                    nc.tensor.matmul(
                        py,
                        oT[oc][:, i * 128:(i + 1) * 128],
                        wo_sb[oc][:, jc * 512:(jc + 1) * 512],
                        start=(oc == 0), stop=(oc == 1))
                nc.vector.tensor_copy(ys[:, jc * 512:(jc + 1) * 512], py)
            nc.sync.dma_start(y_d[i * 128:(i + 1) * 128, :], ys)

    if True:
        for c in range(4):  # chunks of 512 tokens
            tch = slice(c * 512, (c + 1) * 512)
            # ---- projections for chunk c ----
            # Q^T / K^T: [o on partitions, t free]
            for w_sb, b_sb, dest in ((wq_sb, bq_sb, qT), (wk_sb, bk_sb, kT)):
                for oc in range(2):
                    ps = ppr_p.tile([128, 512], f32, name="ps_qk", tag="ps")
                    for kk in range(8):
                        nc.tensor.matmul(
                            ps,
                            w_sb[:, kk * O + oc * 128:kk * O + (oc + 1) * 128],
                            xT[kk][:, tch],
                            start=(kk == 0), stop=(kk == 7))
                    nc.vector.tensor_scalar_add(
                        dest[oc][:, tch], ps, b_sb[:, oc:oc + 1])
            # V natural: [t on partitions, o free]; two 128-token tiles per
            # psum alloc; no bias (folded on host)
            for ah in range(2):
                ps = ppr_p.tile([128, 512], f32, name="ps_v", tag="ps")
                for a2 in range(2):
                    a = 2 * ah + a2
                    for kk in range(8):
                        nc.tensor.matmul(
                            ps[:, a2 * O:(a2 + 1) * O],
                            xT[kk][:, c * 512 + a * 128:c * 512 + (a + 1) * 128],
                            wv_sb[:, kk * O:(kk + 1) * O],
                            start=(kk == 0), stop=(kk == 7))
                for a2 in range(2):
                    a = 2 * ah + a2
                    nc.vector.tensor_copy(
                        vv[c * 4 + a].rearrange(
                            "p (h x) -> p h x", h=HPC)[:, :, 0:S],
                        ps.rearrange("p (a2 h x) -> p a2 h x",
                                     a2=2, h=HPC)[:, a2])

            # ---- attention for q-chunk c ----
            for oc in range(2):  # head pair (2*oc, 2*oc+1)
                po = [pso_p.tile([S + 1, 512], f32, name=f"po{e}",
                                 tag=f"po{e}") for e in range(2)]
                nr = 4 * (c + 1)  # causal: t_k tiles 0..4c+3
                for r in range(nr):
                    m = r - 4 * c
                    j0 = 128 * m if m > 0 else 0  # fully-masked cols skipped
                    ps = pss_p.tile([128, 1024], f32, name="ps_s", tag="pss")
                    for e in range(2):
                        hb = e * 64
                        nc.tensor.matmul(
                            ps[:, e * 512 + j0:(e + 1) * 512],
                            kT[oc][hb:hb + 64, r * 128:(r + 1) * 128],
                            qT[oc][hb:hb + 64, c * 512 + j0:(c + 1) * 512],
                            start=True, stop=True)
                    pt = pt_p.tile([128, 1024], BF16, name="pt_exp",
                                   tag="ptl", bufs=8)
                    ps3 = ps.rearrange("p (e j) -> p e j", e=2)[:, :, j0:]
                    pt3 = pt.rearrange("p (e j) -> p e j", e=2)[:, :, j0:]
                    nc.scalar.activation(pt3, ps3, AF.Exp, scale=inv_scale)
                    if m >= 0:
                        # only the first 128 trimmed columns are partial
                        nc.vector.tensor_mul(
                            pt3[:, :, 0:128], pt3[:, :, 0:128],
                            trimask.rearrange("p (e j) -> p e j", e=2))
                    for e in range(2):
                        h = 2 * oc + e
                        nc.tensor.matmul(
                            po[e][:, j0:],
                            vv[r][:, h * (S + 1):(h + 1) * (S + 1)],
                            pt[:, e * 512 + j0:(e + 1) * 512],
                            start=(r == 0), stop=(r == nr - 1))
                # evict unnormalized O^T rows + denominator row via bf16
                # staging (DMA cannot read PSUM)
                for e in range(2):
                    hb = e * 64
                    stg = pt_p.tile([S + 1, 512], BF16, name=f"ostg{e}",
                                    tag=f"ostg{e}", bufs=3)
                    nc.vector.tensor_copy(stg, po[e])
                    nc.gpsimd.dma_start(oT[oc][hb:hb + 64, tch], stg[0:S, :])
                    nc.sync.dma_start(
                        rsum[e:e + 1, oc * T + c * 512:oc * T + (c + 1) * 512],
                        stg[S:S + 1, :])

            if c > 0:
                phase3(c - 1, ppr_p, pt_p)
        phase3(3, ppr_p, pt_p)


def build_program(loop_n=0):
    nc = bacc.Bacc("TRN2", target_bir_lowering=False, debug=False,
                   num_devices=NCORES)
    d = {
        "x": nc.dram_tensor("xT", [K, T], BF16, kind="ExternalInput").ap(),
        "wqT": nc.dram_tensor("wqT", [K, O], BF16, kind="ExternalInput").ap(),
        "wkT": nc.dram_tensor("wkT", [K, O], BF16, kind="ExternalInput").ap(),
        "wvT": nc.dram_tensor("wvT", [K, O], BF16, kind="ExternalInput").ap(),
        "woT": nc.dram_tensor("woT", [O, K], BF16, kind="ExternalInput").ap(),
        "bq": nc.dram_tensor("bq", [O], F32, kind="ExternalInput").ap(),
        "bk": nc.dram_tensor("bk", [O], F32, kind="ExternalInput").ap(),
        "y": nc.dram_tensor("y", [T, K], BF16, kind="ExternalOutput").ap(),
    }
    with tile.TileContext(nc) as tc:
        _build_body(nc, tc, d, loop_n=loop_n)
    nc.compile()
    return nc


def _get_program():
    if "nc" not in _CACHE:
        _CACHE["nc"] = build_program()
    return _CACHE["nc"]


def make_in_maps(x, Wq_w, Wk_w, Wv_w, Wo_w, Wq_b, Wk_b, Wv_b):
    in_maps = []
    for core in range(NCORES):
        b, hg = divmod(core, 4)
        sl = slice(hg * O, (hg + 1) * O)
        in_maps.append({
            "xT": np.ascontiguousarray(np.asarray(x[b]).T).astype(BF16NP),
            "wqT": np.ascontiguousarray(np.asarray(Wq_w)[sl, :].T).astype(BF16NP),
            "wkT": np.ascontiguousarray(np.asarray(Wk_w)[sl, :].T).astype(BF16NP),
            "wvT": np.ascontiguousarray(np.asarray(Wv_w)[sl, :].T).astype(BF16NP),
            "woT": np.ascontiguousarray(np.asarray(Wo_w)[:, sl].T).astype(BF16NP),
            "bq": np.ascontiguousarray(np.asarray(Wq_b)[sl], np.float32),
            "bk": np.ascontiguousarray(np.asarray(Wk_b)[sl], np.float32),
        })
    return in_maps


def _combine(results, Wv_b, Wo_w, Wo_b):
    bias_row = (np.asarray(Wv_b, np.float32) @ np.asarray(Wo_w, np.float32).T
                + np.asarray(Wo_b, np.float32))
    y = np.empty((B, T, K), np.float32)
    for b in range(B):
        acc = np.asarray(results[b * 4]["y"], np.float32)
        for hg in range(1, 4):
            acc = acc + np.asarray(results[b * 4 + hg]["y"], np.float32)
        y[b] = acc + bias_row
    return y


def kernel(x, Wq_w, Wq_b, Wk_w, Wk_b, Wv_w, Wv_b, Wo_w, Wo_b):
    x = np.asarray(x, np.float32)
    nc = _get_program()
    in_maps = make_in_maps(x, Wq_w, Wk_w, Wv_w, Wo_w, Wq_b, Wk_b, Wv_b)
    out = run_bass_kernel_spmd(nc, in_maps, list(range(NCORES)))
    return _combine(out.results, Wv_b, Wo_w, Wo_b)


# revision 6
# speedup vs baseline: 1.1873x; 1.0256x over previous
"""Trainium2 Bass kernel v2 for nn_MultiHeadAttention_47485158424810.

Sharding (8 cores): core = b*4 + hg — data parallel over batch b, tensor
parallel over 4 head-groups (4 heads x 64 dims = 256 out dims per core).
Each core emits a partial [2048, 1024] f32 output; host sums 4 partials
per batch and adds (Wv_b @ Wo_w.T + Wo_b) — the V-bias is folded on host
since softmax rows sum to one.

vs the f32r baseline (275us): bf16 matmul operands everywhere with f32
PSUM accumulation (FWL weight loads, 2-4x DVE throughput, half the
DMA bytes), denominators packed [2, 2T] so one cheap reciprocal per
chunk replaces four single-partition ones, the rank-1 denominator
broadcast done as a single contraction-2 matmul per (chunk, pair),
V-bias folded into the host combine, per-chunk phase-3 (normalize +
output projection + y DMA) interleaved under the next chunk's
attention via a shared PSUM pool tag (8 banks exactly), and fewer,
queue-balanced DMA dispatches, with deep staging pools (pt x8,
stg x3, ys x3) for pipeline elasticity, and per-pair phase-3
normalize chains that shorten the last chunk's tail.  Measured
174.5us on HW (loop-delta), rel err 5.0e-3.
"""

import os
import sys

import numpy as np

for _p in ("/root/.axon_site/_ro/trn_rl_repo", "/opt/trn_rl_repo"):
    if os.path.isdir(_p) and _p not in sys.path:
        sys.path.append(_p)

import ml_dtypes

import concourse.bass as bass
import concourse.tile as tile
from concourse import bacc, mybir
from concourse.bass_utils import run_bass_kernel_spmd

B, T, K, H = 2, 2048, 1024, 16
NCORES = 8
O = 256  # head-group width per core (4 heads x 64)
S = 64  # head dim
HPC = 4  # heads per core
F32 = mybir.dt.float32
F32R = mybir.dt.float32r
BF16 = mybir.dt.bfloat16
AF = mybir.ActivationFunctionType
ALU = mybir.AluOpType
BF16NP = ml_dtypes.bfloat16

_CACHE = {}


def _build_body(nc, tc, d, loop_n=0):
    # pools live OUTSIDE the timing loop: iterations then pipeline into
    # each other instead of draining at per-iteration pool teardown
    with tc.tile_pool(name="consts", bufs=1) as consts, \
         tc.tile_pool(name="persist", bufs=1) as persist, \
         tc.tile_pool(name="pss", bufs=2, space="PSUM") as pss_p, \
         tc.tile_pool(name="ppr", bufs=2, space="PSUM") as ppr_p, \
         tc.tile_pool(name="pso", bufs=1, space="PSUM") as pso_p, \
         tc.tile_pool(name="ptile", bufs=6) as pt_p:
        pools = (consts, persist, pss_p, ppr_p, pso_p, pt_p)
        if loop_n:
            with tc.For_i(0, loop_n, 1):
                _build_inner(nc, tc, d, *pools)
        else:
            _build_inner(nc, tc, d, *pools)


def _build_inner(nc, tc, d, consts, persist, pss_p, ppr_p, pso_p, pt_p):
    f32 = F32
    x_d, wq_d, wk_d, wv_d, wo_d, bq_d, bk_d, y_d = (
        d["x"], d["wqT"], d["wkT"], d["wvT"], d["woT"],
        d["bq"], d["bk"], d["y"],
    )

    # ---- constant init first: no DMA deps, fills engines while DMAs land
    # causal partial-tile mask: [128, 2x128] bf16 (twin halves per head pair)
    trimask = consts.tile([128, 256], BF16, name="trimask")
    nc.gpsimd.memset(trimask, 1.0)
    tm3 = trimask.rearrange("p (e j) -> p e j", e=2)
    nc.gpsimd.affine_select(
        out=tm3, in_=tm3, pattern=[[0, 2], [1, 128]],
        compare_op=ALU.is_ge, fill=0.0, base=0, channel_multiplier=-1)

    # head-pair selector for the rank-1 denominator broadcast:
    # sel2[0, 0:64] = 1, sel2[1, 64:128] = 1
    sel2_f = consts.tile([2, 128], f32, name="sel2_f")
    sel2 = consts.tile([2, 128], BF16, name="sel2")
    nc.gpsimd.memset(sel2_f, 1.0)
    s3 = sel2_f.rearrange("p (e j) -> p e j", e=2)
    nc.gpsimd.affine_select(
        out=s3, in_=s3, pattern=[[1, 2], [0, 64]],
        compare_op=ALU.is_equal, fill=0.0, base=0, channel_multiplier=-1)
    nc.vector.tensor_copy(sel2, sel2_f)

    # ---- weights: one 3D DMA per tensor, packed [128, 8*256] (k-major) ----
    def load_w(ap_d, nm, eng):
        t_ = consts.tile([128, 8 * O], BF16, name=nm)
        eng.dma_start(t_.rearrange("p (kk o) -> p kk o", kk=8),
                      ap_d.rearrange("(kk p) o -> p kk o", p=128))
        return t_

    wq_sb = load_w(wq_d, "wq", nc.scalar)
    wk_sb = load_w(wk_d, "wk", nc.scalar)
    wv_sb = load_w(wv_d, "wv", nc.scalar)
    wo_sb = []
    for oc in range(2):
        t_ = consts.tile([128, K], BF16, name=f"wo{oc}")
        nc.gpsimd.dma_start(t_, wo_d[oc * 128:(oc + 1) * 128, :])
        wo_sb.append(t_)

    def load_bias(ap_d, nm):
        t_ = consts.tile([128, 2], f32, name=nm)
        nc.gpsimd.dma_start(t_, ap_d.rearrange("(c p) -> p c", p=128))
        return t_

    bq_sb = load_bias(bq_d, "bq_sb")
    bk_sb = load_bias(bk_d, "bk_sb")

    # x^T, full T per k-slice.  Chunk-0 slices first (they gate the first
    # projection), then the remainder.
    xT = [persist.tile([128, T], BF16, name=f"xT{kk}") for kk in range(8)]
    qs = (nc.sync, nc.gpsimd)
    for kk in range(8):
        qs[kk % 2].dma_start(xT[kk][:, 0:512],
                             x_d[kk * 128:(kk + 1) * 128, 0:512])
    for kk in range(8):
        qs[kk % 2].dma_start(xT[kk][:, 512:T],
                             x_d[kk * 128:(kk + 1) * 128, 512:T])

    # persistent activations
    qT = [persist.tile([128, T], BF16, name=f"qT{oc}") for oc in range(2)]
    kT = [persist.tile([128, T], BF16, name=f"kT{oc}") for oc in range(2)]
    oT = [persist.tile([128, T], BF16, name=f"oT{oc}") for oc in range(2)]
    # V natural layout per 128-token tile: 4 heads x (64 dims + ones col)
    vv = [persist.tile([128, HPC * (S + 1)], BF16, name=f"v{i}")
          for i in range(T // 128)]
    # softmax denominators / reciprocals: row e (head-in-pair), col oc*T + t
    rsum = persist.tile([2, 2 * T], BF16, name="rsum")
    rrec = persist.tile([2, 2 * T], BF16, name="rrec")
    ones_f32 = persist.tile([128, 4], f32, name="ones_f32")
    nc.gpsimd.memset(ones_f32, 1.0)
    for i in range(T // 128):
        # ones column at offset h*(S+1)+S for each head
        nc.vector.tensor_copy(vv[i][:, S::S + 1], ones_f32)

    inv_scale = 1.0 / float(np.sqrt(K))

    # steady-state loop: proj(c) -> attention(c) -> phase3(c-1), with the
    # Tile scheduler overlapping phase3(c-1) + proj(c+1) under attention.
    # PSUM: pss 2x[128,1024] (4 banks) + po 2x[65,512] (2) + ppr shared
    # proj/prb/py tag (2) = 8 banks exactly.
    def phase3(c, pool, ys_pool):
        # per-oc normalize: oc=0's chain runs while oc=1's attention of the
        # same chunk is still in flight (matters for the last chunk's tail)
        cr = slice(c * 512, (c + 1) * 512)
        for oc in range(2):
            sl = slice(oc * T + c * 512, oc * T + (c + 1) * 512)
            with nc.allow_low_precision(reason="softmax denom reciprocal"):
                nc.vector.reciprocal(rrec[:, sl], rsum[:, sl])
            prb = pool.tile([128, 512], f32, name="prb", tag="ps")
            nc.tensor.matmul(prb, sel2, rrec[:, sl], start=True, stop=True)
            nc.vector.tensor_mul(oT[oc][:, cr], oT[oc][:, cr], prb)
        for i in range(4 * c, 4 * c + 4):
            ys = ys_pool.tile([128, K], BF16, name="ystg", tag="ystg",
                              bufs=3)
            for jc in range(2):
                py = pool.tile([128, 512], f32, name="py", tag="ps")
                for oc in range(2):
                    nc.tensor.matmul(
                        py,
                        oT[oc][:, i * 128:(i + 1) * 128],
                        wo_sb[oc][:, jc * 512:(jc + 1) * 512],
                        start=(oc == 0), stop=(oc == 1))
                nc.vector.tensor_copy(ys[:, jc * 512:(jc + 1) * 512], py)
            nc.sync.dma_start(y_d[i * 128:(i + 1) * 128, :], ys)

    if True:
        for c in range(4):  # chunks of 512 tokens
            tch = slice(c * 512, (c + 1) * 512)
            # ---- projections for chunk c ----
            # Q^T / K^T: [o on partitions, t free]
            for w_sb, b_sb, dest in ((wq_sb, bq_sb, qT), (wk_sb, bk_sb, kT)):
                for oc in range(2):
                    ps = ppr_p.tile([128, 512], f32, name="ps_qk", tag="ps")
                    for kk in range(8):
                        nc.tensor.matmul(
                            ps,
                            w_sb[:, kk * O + oc * 128:kk * O + (oc + 1) * 128],
                            xT[kk][:, tch],
                            start=(kk == 0), stop=(kk == 7))
                    nc.vector.tensor_scalar_add(
                        dest[oc][:, tch], ps, b_sb[:, oc:oc + 1])
            # V natural: [t on partitions, o free]; two 128-token tiles per
            # psum alloc; no bias (folded on host)
            for ah in range(2):
                ps = ppr_p.tile([128, 512], f32, name="ps_v", tag="ps")
                for a2 in range(2):
                    a = 2 * ah + a2
                    for kk in range(8):
                        nc.tensor.matmul(
                            ps[:, a2 * O:(a2 + 1) * O],
                            xT[kk][:, c * 512 + a * 128:c * 512 + (a + 1) * 128],
                            wv_sb[:, kk * O:(kk + 1) * O],
                            start=(kk == 0), stop=(kk == 7))
                for a2 in range(2):
                    a = 2 * ah + a2
                    nc.vector.tensor_copy(
                        vv[c * 4 + a].rearrange(
                            "p (h x) -> p h x", h=HPC)[:, :, 0:S],
                        ps.rearrange("p (a2 h x) -> p a2 h x",
                                     a2=2, h=HPC)[:, a2])

            # ---- attention for q-chunk c ----
            for oc in range(2):  # head pair (2*oc, 2*oc+1)
                po = [pso_p.tile([S + 1, 512], f32, name=f"po{e}",
                                 tag=f"po{e}") for e in range(2)]
                nr = 4 * (c + 1)  # causal: t_k tiles 0..4c+3
                for r in range(nr):
                    m = r - 4 * c
                    j0 = 128 * m if m > 0 else 0  # fully-masked cols skipped
                    ps = pss_p.tile([128, 1024], f32, name="ps_s", tag="pss")
                    for e in range(2):
                        hb = e * 64
                        nc.tensor.matmul(
                            ps[:, e * 512 + j0:(e + 1) * 512],
                            kT[oc][hb:hb + 64, r * 128:(r + 1) * 128],
                            qT[oc][hb:hb + 64, c * 512 + j0:(c + 1) * 512],
                            start=True, stop=True)
                    pt = pt_p.tile([128, 1024], BF16, name="pt_exp",
                                   tag="ptl", bufs=8)
                    ps3 = ps.rearrange("p (e j) -> p e j", e=2)[:, :, j0:]
                    pt3 = pt.rearrange("p (e j) -> p e j", e=2)[:, :, j0:]
                    nc.scalar.activation(pt3, ps3, AF.Exp, scale=inv_scale)
                    if m >= 0:
                        # only the first 128 trimmed columns are partial
                        nc.vector.tensor_mul(
                            pt3[:, :, 0:128], pt3[:, :, 0:128],
                            trimask.rearrange("p (e j) -> p e j", e=2))
                    for e in range(2):
                        h = 2 * oc + e
                        nc.tensor.matmul(
                            po[e][:, j0:],
                            vv[r][:, h * (S + 1):(h + 1) * (S + 1)],
                            pt[:, e * 512 + j0:(e + 1) * 512],
                            start=(r == 0), stop=(r == nr - 1))
                # evict unnormalized O^T rows + denominator row via bf16
                # staging (DMA cannot read PSUM)
                for e in range(2):
                    hb = e * 64
                    stg = pt_p.tile([S + 1, 512], BF16, name=f"ostg{e}",
                                    tag=f"ostg{e}", bufs=3)
                    nc.vector.tensor_copy(stg, po[e])
                    nc.gpsimd.dma_start(oT[oc][hb:hb + 64, tch], stg[0:S, :])
                    nc.sync.dma_start(
                        rsum[e:e + 1, oc * T + c * 512:oc * T + (c + 1) * 512],
                        stg[S:S + 1, :])

            if c > 0:
                phase3(c - 1, ppr_p, pt_p)
        phase3(3, ppr_p, pt_p)


def build_program(loop_n=0):
    nc = bacc.Bacc("TRN2", target_bir_lowering=False, debug=False,
                   num_devices=NCORES)
    d = {
        "x": nc.dram_tensor("xT", [K, T], BF16, kind="ExternalInput").ap(),
        "wqT": nc.dram_tensor("wqT", [K, O], BF16, kind="ExternalInput").ap(),
        "wkT": nc.dram_tensor("wkT", [K, O], BF16, kind="ExternalInput").ap(),
        "wvT": nc.dram_tensor("wvT", [K, O], BF16, kind="ExternalInput").ap(),
        "woT": nc.dram_tensor("woT", [O, K], BF16, kind="ExternalInput").ap(),
        "bq": nc.dram_tensor("bq", [O], F32, kind="ExternalInput").ap(),
        "bk": nc.dram_tensor("bk", [O], F32, kind="ExternalInput").ap(),
        "y": nc.dram_tensor("y", [T, K], BF16, kind="ExternalOutput").ap(),
    }
    with tile.TileContext(nc) as tc:
        _build_body(nc, tc, d, loop_n=loop_n)
    nc.compile()
    return nc


def _get_program():
    if "nc" not in _CACHE:
        _CACHE["nc"] = build_program()
    return _CACHE["nc"]


def make_in_maps(x, Wq_w, Wk_w, Wv_w, Wo_w, Wq_b, Wk_b, Wv_b):
    in_maps = []
    for core in range(NCORES):
        b, hg = divmod(core, 4)
        sl = slice(hg * O, (hg + 1) * O)
        in_maps.append({
            "xT": np.ascontiguousarray(np.asarray(x[b]).T).astype(BF16NP),
            "wqT": np.ascontiguousarray(np.asarray(Wq_w)[sl, :].T).astype(BF16NP),
            "wkT": np.ascontiguousarray(np.asarray(Wk_w)[sl, :].T).astype(BF16NP),
            "wvT": np.ascontiguousarray(np.asarray(Wv_w)[sl, :].T).astype(BF16NP),
            "woT": np.ascontiguousarray(np.asarray(Wo_w)[:, sl].T).astype(BF16NP),
            "bq": np.ascontiguousarray(np.asarray(Wq_b)[sl], np.float32),
            "bk": np.ascontiguousarray(np.asarray(Wk_b)[sl], np.float32),
        })
    return in_maps


def _combine(results, Wv_b, Wo_w, Wo_b):
    bias_row = (np.asarray(Wv_b, np.float32) @ np.asarray(Wo_w, np.float32).T
                + np.asarray(Wo_b, np.float32))
    y = np.empty((B, T, K), np.float32)
    for b in range(B):
        acc = np.asarray(results[b * 4]["y"], np.float32)
        for hg in range(1, 4):
            acc = acc + np.asarray(results[b * 4 + hg]["y"], np.float32)
        y[b] = acc + bias_row
    return y


def kernel(x, Wq_w, Wq_b, Wk_w, Wk_b, Wv_w, Wv_b, Wo_w, Wo_b):
    x = np.asarray(x, np.float32)
    nc = _get_program()
    in_maps = make_in_maps(x, Wq_w, Wk_w, Wv_w, Wo_w, Wq_b, Wk_b, Wv_b)
    out = run_bass_kernel_spmd(nc, in_maps, list(range(NCORES)))
    return _combine(out.results, Wv_b, Wo_w, Wo_b)
